# revision 4
# baseline (speedup 1.0000x reference)
"""nn_KGCN kernel: 3-layer GNN message passing on 8 Trainium2 NeuronCores.

Device (raw Bass, 4-engine pipeline, sharded 8-way): all six dense MLPs
(3 edge-level over 1.6M edges, 3 node-level over 100K nodes), which carry
virtually all FLOPs. Host (numpy): static index gathers x[src]/x[dst],
concat, and dst segment-sum (graph-structure bookkeeping on int indices).
"""
import sys
sys.path.insert(0, '/opt/trn_rl_repo')
from contextlib import ExitStack

import numpy as np
import jax
from jax.sharding import Mesh, PartitionSpec
from jax.experimental.shard_map import shard_map

from concourse import bass, mybir
from concourse.bass2jax import _bass_exec_p, install_neuronx_cc_hook, partition_id_tensor

F32 = mybir.dt.float32
N_CORES = 8
DIN = 48      # padded MLP input width (max over layers: 16+16+16)
DH = 16       # hidden/output width (out padded to 16)
CH = 512      # free-dim chunk per matmul / PSUM tile


class _SpmdRunner:
    """Compile a Bass program once; run it repeatedly on 8 cores via PJRT."""

    def __init__(self, nc, n_cores=N_CORES):
        install_neuronx_cc_hook()
        self.n_cores = n_cores
        pname = nc.partition_id_tensor.name if nc.partition_id_tensor else None
        in_names, out_names, out_avals, zero_outs = [], [], [], []
        for alloc in nc.m.functions[0].allocations:
            if not isinstance(alloc, mybir.MemoryLocationSet):
                continue
            name = alloc.memorylocations[0].name
            if alloc.kind == "ExternalInput":
                if name != pname:
                    in_names.append(name)
            elif alloc.kind == "ExternalOutput":
                shape = tuple(alloc.tensor_shape)
                dtype = mybir.dt.np(alloc.dtype)
                out_names.append(name)
                out_avals.append(jax.core.ShapedArray(shape, dtype))
                zero_outs.append(np.zeros(shape, dtype))
        self.in_names, self.out_names = in_names, out_names
        self.out_avals, self.zero_outs = out_avals, zero_outs
        n_params, n_outs = len(in_names), len(out_avals)
        self.n_params = n_params
        all_in = in_names + out_names + ([pname] if pname else [])

        def _body(*args):
            operands = list(args)
            if pname is not None:
                operands.append(partition_id_tensor())
            return tuple(_bass_exec_p.bind(
                *operands, out_avals=tuple(out_avals), in_names=tuple(all_in),
                out_names=tuple(out_names), lowering_input_output_aliases=(),
                sim_require_finite=True, sim_require_nnan=True, nc=nc))

        devices = jax.devices()[:n_cores]
        mesh = Mesh(np.asarray(devices), ("core",))
        self.sharded = jax.jit(
            shard_map(_body, mesh=mesh,
                      in_specs=(PartitionSpec("core"),) * (n_params + n_outs),
                      out_specs=(PartitionSpec("core"),) * n_outs,
                      check_rep=False),
            donate_argnums=tuple(range(n_params, n_params + n_outs)),
            keep_unused=True)

    def __call__(self, in_maps):
        per_core = [[np.ascontiguousarray(m[n]) for n in self.in_names] for m in in_maps]
        concat_in = [np.concatenate([per_core[c][i] for c in range(self.n_cores)], axis=0)
                     for i in range(self.n_params)]
        concat_zeros = [np.zeros((self.n_cores * z.shape[0], *z.shape[1:]), z.dtype)
                        for z in self.zero_outs]
        outs = [np.asarray(a) for a in jax.block_until_ready(self.sharded(*concat_in, *concat_zeros))]
        return [{n: outs[i].reshape(self.n_cores, *self.out_avals[i].shape)[c]
                 for i, n in enumerate(self.out_names)} for c in range(self.n_cores)]


def _build_mlp_program(E):
    """h = relu(x@W1+b1); h = relu(h@W2+b2); out = h@W3+b3   (feat-major).

    xT [DIN, E] -> outT [DH, E]. E must be a multiple of CH.
    Four-engine software pipeline: SP does DMA, PE matmuls, ACT the two
    relu+bias epilogues, DVE the final bias add (no relu).
    """
    assert E % CH == 0
    NCHUNK = E // CH
    nc = bass.Bass()
    xT = nc.declare_dram_parameter("xT", [DIN, E], F32, isOutput=False)
    w1 = nc.declare_dram_parameter("w1", [DIN, DH], F32, isOutput=False)
    w2 = nc.declare_dram_parameter("w2", [DH, DH], F32, isOutput=False)
    w3 = nc.declare_dram_parameter("w3", [DH, DH], F32, isOutput=False)
    b1 = nc.declare_dram_parameter("b1", [DH, 1], F32, isOutput=False)
    b2 = nc.declare_dram_parameter("b2", [DH, 1], F32, isOutput=False)
    b3 = nc.declare_dram_parameter("b3", [DH, 1], F32, isOutput=False)
    outT = nc.declare_dram_parameter("outT", [DH, E], F32, isOutput=True)

    ctx = ExitStack()
    w1s = ctx.enter_context(nc.sbuf_tensor("w1s", [DIN, DH], F32))
    w2s = ctx.enter_context(nc.sbuf_tensor("w2s", [DH, DH], F32))
    w3s = ctx.enter_context(nc.sbuf_tensor("w3s", [DH, DH], F32))
    b1s = ctx.enter_context(nc.sbuf_tensor("b1s", [DH, 1], F32))
    b2s = ctx.enter_context(nc.sbuf_tensor("b2s", [DH, 1], F32))
    b3s = ctx.enter_context(nc.sbuf_tensor("b3s", [DH, 1], F32))
    xin = [ctx.enter_context(nc.sbuf_tensor(f"xin{k}", [DIN, CH], F32)) for k in range(2)]
    h1 = [ctx.enter_context(nc.sbuf_tensor(f"h1_{k}", [DH, CH], F32)) for k in range(2)]
    h2 = [ctx.enter_context(nc.sbuf_tensor(f"h2_{k}", [DH, CH], F32)) for k in range(2)]
    osb = [ctx.enter_context(nc.sbuf_tensor(f"osb{k}", [DH, CH], F32)) for k in range(2)]
    ps1 = ctx.enter_context(nc.psum_tensor("ps1", [DH, CH], F32))
    ps2 = ctx.enter_context(nc.psum_tensor("ps2", [DH, CH], F32))
    ps3 = ctx.enter_context(nc.psum_tensor("ps3", [DH, CH], F32))

    with ctx, nc.Block() as block, \
            nc.semaphore("wsem") as wsem, nc.semaphore("dsem") as dsem, \
            nc.semaphore("tsem") as tsem, nc.semaphore("ssem") as ssem, \
            nc.semaphore("vsem") as vsem, nc.semaphore("osem") as osem:

        @block.sync
        def _(sp):
            for src, dst in ((w1, w1s), (w2, w2s), (w3, w3s),
                             (b1, b1s), (b2, b2s), (b3, b3s)):
                sp.dma_start(out=dst[:], in_=src[:]).then_inc(wsem, 16)
            for i in range(NCHUNK):
                if i >= 2:
                    # xin[i%2] was last read by MM1 of chunk i-2
                    sp.wait_ge(tsem, 3 * (i - 2) + 1)
                sp.dma_start(out=xin[i % 2][:],
                             in_=xT[:, i * CH:(i + 1) * CH]).then_inc(dsem, 16)
                if i >= 1:
                    sp.wait_ge(vsem, i)
                    sp.dma_start(out=outT[:, (i - 1) * CH:i * CH],
                                 in_=osb[(i - 1) % 2][:]).then_inc(osem, 16)
            sp.wait_ge(vsem, NCHUNK)
            sp.dma_start(out=outT[:, (NCHUNK - 1) * CH:NCHUNK * CH],
                         in_=osb[(NCHUNK - 1) % 2][:]).then_inc(osem, 16)

        @block.tensor
        def _(pe):
            pe.wait_ge(wsem, 96)
            for i in range(NCHUNK):
                pe.wait_ge(dsem, 16 * (i + 1))
                if i >= 1:
                    pe.wait_ge(ssem, 2 * (i - 1) + 1)   # ps1 free (relu1 of i-1 done)
                pe.matmul(out=ps1[:], lhsT=w1s[:], rhs=xin[i % 2][:],
                          start=True, stop=True).then_inc(tsem, 1)
                pe.wait_ge(ssem, 2 * i + 1)             # h1[i%2] ready
                pe.matmul(out=ps2[:], lhsT=w2s[:], rhs=h1[i % 2][:],
                          start=True, stop=True).then_inc(tsem, 1)
                pe.wait_ge(ssem, 2 * i + 2)             # h2[i%2] ready
                if i >= 1:
                    pe.wait_ge(vsem, i)                 # ps3 free
                pe.matmul(out=ps3[:], lhsT=w3s[:], rhs=h2[i % 2][:],
                          start=True, stop=True).then_inc(tsem, 1)

        @block.scalar
        def _(act):
            act.wait_ge(wsem, 96)
            for i in range(NCHUNK):
                act.wait_ge(tsem, 3 * i + 1)
                if i >= 2:
                    act.wait_ge(tsem, 3 * (i - 2) + 2)  # h1[i%2] free (MM2 of i-2)
                act.activation(out=h1[i % 2][:], in_=ps1[:],
                               func=mybir.ActivationFunctionType.Relu,
                               bias=b1s[:, :1]).then_inc(ssem, 1)
                act.wait_ge(tsem, 3 * i + 2)
                if i >= 2:
                    act.wait_ge(tsem, 3 * (i - 2) + 3)  # h2[i%2] free (MM3 of i-2)
                act.activation(out=h2[i % 2][:], in_=ps2[:],
                               func=mybir.ActivationFunctionType.Relu,
                               bias=b2s[:, :1]).then_inc(ssem, 1)

        @block.vector
        def _(dve):
            dve.wait_ge(wsem, 96)
            for i in range(NCHUNK):
                dve.wait_ge(tsem, 3 * i + 3)
                if i >= 2:
                    dve.wait_ge(osem, 16 * (i - 1))     # osb[i%2] free (out-DMA i-2)
                dve.tensor_tensor(out=osb[i % 2][:], in0=ps3[:],
                                  in1=b3s[:, :1].to_broadcast([DH, CH]),
                                  op=mybir.AluOpType.add).then_inc(vsem, 1)
    return nc


_RUNNERS = {}


def _mlp_runner(E):
    if E not in _RUNNERS:
        _RUNNERS[E] = _SpmdRunner(_build_mlp_program(E))
    return _RUNNERS[E]


def _pad_to(x, n, axis=0):
    pad = [(0, 0)] * x.ndim
    pad[axis] = (0, n - x.shape[axis])
    return np.pad(x, pad) if n > x.shape[axis] else x


def _device_mlp(X, layers):
    """X [N, din] -> MLP over padded [DIN->DH->DH->DH] on 8 cores.

    layers: list of (W, b) numpy, 2 or 3 layers; relu after first two GEMMs,
    none after the third. 2-layer MLPs (relu on both) pass identity as GEMM3.
    """
    N, din = X.shape
    assert din <= DIN
    (W1, bb1), (W2, bb2) = layers[0], layers[1]
    if len(layers) == 3:
        W3, bb3 = layers[2]
    else:
        W3, bb3 = np.eye(DH, dtype=np.float32), np.zeros((DH,), np.float32)
    w1 = _pad_to(_pad_to(np.asarray(W1, np.float32), DIN, 0), DH, 1)
    w2 = _pad_to(np.asarray(W2, np.float32), DH, 1)
    w3 = _pad_to(np.asarray(W3, np.float32), DH, 1)
    b1 = _pad_to(np.asarray(bb1, np.float32), DH)[:, None]
    b2 = _pad_to(np.asarray(bb2, np.float32), DH)[:, None]
    b3 = _pad_to(np.asarray(bb3, np.float32), DH)[:, None]

    per = -(-N // N_CORES)           # rows per core
    per = -(-per // CH) * CH         # multiple of CH
    E = per
    runner = _mlp_runner(E)
    Xp = _pad_to(np.asarray(X, np.float32), N_CORES * per, 0)
    Xp = _pad_to(Xp, DIN, 1)
    in_maps = []
    for c in range(N_CORES):
        xT = np.ascontiguousarray(Xp[c * per:(c + 1) * per].T)
        in_maps.append({"xT": xT, "w1": w1, "w2": w2, "w3": w3,
                        "b1": b1, "b2": b2, "b3": b3})
    res = runner(in_maps)
    out = np.concatenate([res[c]["outT"].T for c in range(N_CORES)], axis=0)
    return out[:N]


def kernel(x_node, x_edge, edge_index, params):
    x_node = np.asarray(x_node, np.float32)
    x_edge = np.asarray(x_edge, np.float32)
    edge_index = np.asarray(edge_index)
    src, dst = edge_index[0], edge_index[1]
    n = x_node.shape[0]

    # dst-sorted order so segment-sum is a fast reduceat on host
    order = np.argsort(dst, kind="stable")
    src_s, dst_s = src[order], dst[order]
    starts = np.searchsorted(dst_s, np.arange(n))
    starts_c = np.minimum(starts, len(dst_s) - 1)
    deg = np.bincount(dst_s, minlength=n)

    def seg_sum(msg_s):
        out = np.add.reduceat(msg_s, starts_c, axis=0)
        out[deg == 0] = 0.0
        return out

    def conv(x, e, edge_p, node_p, nlayers_e, nlayers_n):
        ein = np.concatenate([x[src_s], e[order], x[dst_s]], axis=1)
        e_new = _device_mlp(ein, edge_p)[:, :edge_p[-1][0].shape[1]]
        msg = np.concatenate([x[src_s], e_new], axis=1)
        agg = seg_sum(msg)
        x_new = _device_mlp(agg, node_p)[:, :node_p[-1][0].shape[1]]
        # e_new is in sorted order; restore original edge order
        e_out = np.empty_like(e_new)
        e_out[order] = e_new
        return x_new, e_out

    p = params
    c1e = [(np.asarray(W), np.asarray(b)) for W, b in p['c1_edge']]
    c1n = [(np.asarray(W), np.asarray(b)) for W, b in p['c1_node']]
    c2e = [(np.asarray(W), np.asarray(b)) for W, b in p['c2_edge']]
    c2n = [(np.asarray(W), np.asarray(b)) for W, b in p['c2_node']]
    c3e = [(np.asarray(W), np.asarray(b)) for W, b in p['c3_edge']]
    c3n = [(np.asarray(W), np.asarray(b)) for W, b in p['c3_node']]

    x, e = conv(x_node, x_edge, c1e, c1n, 2, 2)
    x, e = conv(x, e, c2e, c2n, 2, 2)
    x, e = conv(x, e, c3e, c3n, 3, 3)
    return (x, e)


# revision 5
# speedup vs baseline: 1.2331x; 1.2331x over previous
"""nn_KGCN kernel: 3-layer GNN message passing on 8 Trainium2 NeuronCores.

Device (raw Bass, 4-engine pipeline, sharded 8-way): all six dense MLPs
(3 edge-level over 1.6M edges, 3 node-level over 100K nodes), which carry
virtually all FLOPs. Host (numpy): static index gathers x[src]/x[dst],
concat, and dst segment-sum (graph-structure bookkeeping on int indices).
"""
import sys
sys.path.insert(0, '/opt/trn_rl_repo')
from contextlib import ExitStack

import numpy as np
import jax
from ml_dtypes import bfloat16
from jax.sharding import Mesh, PartitionSpec
from jax.experimental.shard_map import shard_map

from concourse import bass, mybir
from concourse.bass2jax import _bass_exec_p, install_neuronx_cc_hook, partition_id_tensor

F32 = mybir.dt.float32
BF16 = mybir.dt.bfloat16
N_CORES = 8
DIN = 48      # padded MLP input width (max over layers: 16+16+16)
DH = 16       # hidden/output width (out padded to 16)
CH = 512      # free-dim chunk per matmul / PSUM tile


class _SpmdRunner:
    """Compile a Bass program once; run it repeatedly on 8 cores via PJRT."""

    def __init__(self, nc, n_cores=N_CORES):
        install_neuronx_cc_hook()
        self.n_cores = n_cores
        pname = nc.partition_id_tensor.name if nc.partition_id_tensor else None
        in_names, out_names, out_avals, zero_outs = [], [], [], []
        for alloc in nc.m.functions[0].allocations:
            if not isinstance(alloc, mybir.MemoryLocationSet):
                continue
            name = alloc.memorylocations[0].name
            if alloc.kind == "ExternalInput":
                if name != pname:
                    in_names.append(name)
            elif alloc.kind == "ExternalOutput":
                shape = tuple(alloc.tensor_shape)
                dtype = mybir.dt.np(alloc.dtype)
                out_names.append(name)
                out_avals.append(jax.core.ShapedArray(shape, dtype))
                zero_outs.append(np.zeros(shape, dtype))
        self.in_names, self.out_names = in_names, out_names
        self.out_avals, self.zero_outs = out_avals, zero_outs
        n_params, n_outs = len(in_names), len(out_avals)
        self.n_params = n_params
        all_in = in_names + out_names + ([pname] if pname else [])

        def _body(*args):
            operands = list(args)
            if pname is not None:
                operands.append(partition_id_tensor())
            return tuple(_bass_exec_p.bind(
                *operands, out_avals=tuple(out_avals), in_names=tuple(all_in),
                out_names=tuple(out_names), lowering_input_output_aliases=(),
                sim_require_finite=True, sim_require_nnan=True, nc=nc))

        devices = jax.devices()[:n_cores]
        mesh = Mesh(np.asarray(devices), ("core",))
        self.sharded = jax.jit(
            shard_map(_body, mesh=mesh,
                      in_specs=(PartitionSpec("core"),) * (n_params + n_outs),
                      out_specs=(PartitionSpec("core"),) * n_outs,
                      check_rep=False),
            donate_argnums=tuple(range(n_params, n_params + n_outs)),
            keep_unused=True)

    def __call__(self, in_maps):
        per_core = [[np.ascontiguousarray(m[n]) for n in self.in_names] for m in in_maps]
        concat_in = [np.concatenate([per_core[c][i] for c in range(self.n_cores)], axis=0)
                     for i in range(self.n_params)]
        concat_zeros = [np.zeros((self.n_cores * z.shape[0], *z.shape[1:]), z.dtype)
                        for z in self.zero_outs]
        outs = [np.asarray(a) for a in jax.block_until_ready(self.sharded(*concat_in, *concat_zeros))]
        return [{n: outs[i].reshape(self.n_cores, *self.out_avals[i].shape)[c]
                 for i, n in enumerate(self.out_names)} for c in range(self.n_cores)]


def _build_mlp_program(E):
    """h = relu(x@W1+b1); h = relu(h@W2+b2); out = h@W3+b3   (feat-major).

    xT [DIN, E] -> outT [DH, E]. E must be a multiple of CH.
    Four-engine software pipeline: SP does DMA, PE matmuls, ACT the two
    relu+bias epilogues, DVE the final bias add (no relu).
    """
    assert E % CH == 0
    NCHUNK = E // CH
    nc = bass.Bass()
    xT = nc.declare_dram_parameter("xT", [DIN, E], BF16, isOutput=False)
    w1 = nc.declare_dram_parameter("w1", [DIN, DH], BF16, isOutput=False)
    w2 = nc.declare_dram_parameter("w2", [DH, DH], F32, isOutput=False)
    w3 = nc.declare_dram_parameter("w3", [DH, DH], F32, isOutput=False)
    b1 = nc.declare_dram_parameter("b1", [DH, 1], F32, isOutput=False)
    b2 = nc.declare_dram_parameter("b2", [DH, 1], F32, isOutput=False)
    b3 = nc.declare_dram_parameter("b3", [DH, 1], F32, isOutput=False)
    outT = nc.declare_dram_parameter("outT", [DH, E], BF16, isOutput=True)

    ctx = ExitStack()
    w1s = ctx.enter_context(nc.sbuf_tensor("w1s", [DIN, DH], BF16))
    w2s = ctx.enter_context(nc.sbuf_tensor("w2s", [DH, DH], F32))
    w3s = ctx.enter_context(nc.sbuf_tensor("w3s", [DH, DH], F32))
    b1s = ctx.enter_context(nc.sbuf_tensor("b1s", [DH, 1], F32))
    b2s = ctx.enter_context(nc.sbuf_tensor("b2s", [DH, 1], F32))
    b3s = ctx.enter_context(nc.sbuf_tensor("b3s", [DH, 1], F32))
    xin = [ctx.enter_context(nc.sbuf_tensor(f"xin{k}", [DIN, CH], BF16)) for k in range(2)]
    h1 = [ctx.enter_context(nc.sbuf_tensor(f"h1_{k}", [DH, CH], F32)) for k in range(2)]
    h2 = [ctx.enter_context(nc.sbuf_tensor(f"h2_{k}", [DH, CH], F32)) for k in range(2)]
    osb = [ctx.enter_context(nc.sbuf_tensor(f"osb{k}", [DH, CH], BF16)) for k in range(2)]
    ps1 = ctx.enter_context(nc.psum_tensor("ps1", [DH, CH], F32))
    ps2 = ctx.enter_context(nc.psum_tensor("ps2", [DH, CH], F32))
    ps3 = ctx.enter_context(nc.psum_tensor("ps3", [DH, CH], F32))

    with ctx, nc.Block() as block, \
            nc.semaphore("wsem") as wsem, nc.semaphore("dsem") as dsem, \
            nc.semaphore("tsem") as tsem, nc.semaphore("ssem") as ssem, \
            nc.semaphore("vsem") as vsem, nc.semaphore("osem") as osem:

        @block.sync
        def _(sp):
            for src, dst in ((w1, w1s), (w2, w2s), (w3, w3s),
                             (b1, b1s), (b2, b2s), (b3, b3s)):
                sp.dma_start(out=dst[:], in_=src[:]).then_inc(wsem, 16)
            for i in range(NCHUNK):
                if i >= 2:
                    # xin[i%2] was last read by MM1 of chunk i-2
                    sp.wait_ge(tsem, 3 * (i - 2) + 1)
                sp.dma_start(out=xin[i % 2][:],
                             in_=xT[:, i * CH:(i + 1) * CH]).then_inc(dsem, 16)
                if i >= 1:
                    sp.wait_ge(vsem, i)
                    sp.dma_start(out=outT[:, (i - 1) * CH:i * CH],
                                 in_=osb[(i - 1) % 2][:]).then_inc(osem, 16)
            sp.wait_ge(vsem, NCHUNK)
            sp.dma_start(out=outT[:, (NCHUNK - 1) * CH:NCHUNK * CH],
                         in_=osb[(NCHUNK - 1) % 2][:]).then_inc(osem, 16)

        @block.tensor
        def _(pe):
            pe.wait_ge(wsem, 96)
            for i in range(NCHUNK):
                pe.wait_ge(dsem, 16 * (i + 1))
                if i >= 1:
                    pe.wait_ge(ssem, 2 * (i - 1) + 1)   # ps1 free (relu1 of i-1 done)
                pe.matmul(out=ps1[:], lhsT=w1s[:], rhs=xin[i % 2][:],
                          start=True, stop=True).then_inc(tsem, 1)
                pe.wait_ge(ssem, 2 * i + 1)             # h1[i%2] ready
                pe.matmul(out=ps2[:], lhsT=w2s[:], rhs=h1[i % 2][:],
                          start=True, stop=True).then_inc(tsem, 1)
                pe.wait_ge(ssem, 2 * i + 2)             # h2[i%2] ready
                if i >= 1:
                    pe.wait_ge(vsem, i)                 # ps3 free
                pe.matmul(out=ps3[:], lhsT=w3s[:], rhs=h2[i % 2][:],
                          start=True, stop=True).then_inc(tsem, 1)

        @block.scalar
        def _(act):
            act.wait_ge(wsem, 96)
            for i in range(NCHUNK):
                act.wait_ge(tsem, 3 * i + 1)
                if i >= 2:
                    act.wait_ge(tsem, 3 * (i - 2) + 2)  # h1[i%2] free (MM2 of i-2)
                act.activation(out=h1[i % 2][:], in_=ps1[:],
                               func=mybir.ActivationFunctionType.Relu,
                               bias=b1s[:, :1]).then_inc(ssem, 1)
                act.wait_ge(tsem, 3 * i + 2)
                if i >= 2:
                    act.wait_ge(tsem, 3 * (i - 2) + 3)  # h2[i%2] free (MM3 of i-2)
                act.activation(out=h2[i % 2][:], in_=ps2[:],
                               func=mybir.ActivationFunctionType.Relu,
                               bias=b2s[:, :1]).then_inc(ssem, 1)

        @block.vector
        def _(dve):
            dve.wait_ge(wsem, 96)
            for i in range(NCHUNK):
                dve.wait_ge(tsem, 3 * i + 3)
                if i >= 2:
                    dve.wait_ge(osem, 16 * (i - 1))     # osb[i%2] free (out-DMA i-2)
                dve.tensor_tensor(out=osb[i % 2][:], in0=ps3[:],
                                  in1=b3s[:, :1].to_broadcast([DH, CH]),
                                  op=mybir.AluOpType.add).then_inc(vsem, 1)
    return nc


_RUNNERS = {}


def _mlp_runner(E):
    if E not in _RUNNERS:
        _RUNNERS[E] = _SpmdRunner(_build_mlp_program(E))
    return _RUNNERS[E]


def _pad_to(x, n, axis=0):
    pad = [(0, 0)] * x.ndim
    pad[axis] = (0, n - x.shape[axis])
    return np.pad(x, pad) if n > x.shape[axis] else x


def _device_mlp(X, layers):
    """X [N, din] -> MLP over padded [DIN->DH->DH->DH] on 8 cores.

    layers: list of (W, b) numpy, 2 or 3 layers; relu after first two GEMMs,
    none after the third. 2-layer MLPs (relu on both) pass identity as GEMM3.
    """
    N, din = X.shape
    assert din <= DIN
    (W1, bb1), (W2, bb2) = layers[0], layers[1]
    if len(layers) == 3:
        W3, bb3 = layers[2]
    else:
        W3, bb3 = np.eye(DH, dtype=np.float32), np.zeros((DH,), np.float32)
    w1 = _pad_to(_pad_to(np.asarray(W1, np.float32), DIN, 0), DH, 1).astype(bfloat16)
    w2 = _pad_to(np.asarray(W2, np.float32), DH, 1)
    w3 = _pad_to(np.asarray(W3, np.float32), DH, 1)
    b1 = _pad_to(np.asarray(bb1, np.float32), DH)[:, None]
    b2 = _pad_to(np.asarray(bb2, np.float32), DH)[:, None]
    b3 = _pad_to(np.asarray(bb3, np.float32), DH)[:, None]

    per = -(-N // N_CORES)           # rows per core
    per = -(-per // CH) * CH         # multiple of CH
    E = per
    runner = _mlp_runner(E)
    Xp = _pad_to(np.asarray(X, np.float32), N_CORES * per, 0)
    Xp = _pad_to(Xp, DIN, 1)
    in_maps = []
    for c in range(N_CORES):
        xT = np.ascontiguousarray(Xp[c * per:(c + 1) * per].T.astype(bfloat16))
        in_maps.append({"xT": xT, "w1": w1, "w2": w2, "w3": w3,
                        "b1": b1, "b2": b2, "b3": b3})
    res = runner(in_maps)
    out = np.concatenate([res[c]["outT"].T.astype(np.float32) for c in range(N_CORES)], axis=0)
    return out[:N]


def kernel(x_node, x_edge, edge_index, params):
    x_node = np.asarray(x_node, np.float32)
    x_edge = np.asarray(x_edge, np.float32)
    edge_index = np.asarray(edge_index)
    src, dst = edge_index[0], edge_index[1]
    n = x_node.shape[0]

    # dst-sorted order so segment-sum is a fast reduceat on host
    order = np.argsort(dst, kind="stable")
    src_s, dst_s = src[order], dst[order]
    starts = np.searchsorted(dst_s, np.arange(n))
    starts_c = np.minimum(starts, len(dst_s) - 1)
    deg = np.bincount(dst_s, minlength=n)

    def seg_sum(msg_s):
        out = np.add.reduceat(msg_s, starts_c, axis=0)
        out[deg == 0] = 0.0
        return out

    def conv(x, e, edge_p, node_p, nlayers_e, nlayers_n):
        ein = np.concatenate([x[src_s], e[order], x[dst_s]], axis=1)
        e_new = _device_mlp(ein, edge_p)[:, :edge_p[-1][0].shape[1]]
        msg = np.concatenate([x[src_s], e_new], axis=1)
        agg = seg_sum(msg)
        x_new = _device_mlp(agg, node_p)[:, :node_p[-1][0].shape[1]]
        # e_new is in sorted order; restore original edge order
        e_out = np.empty_like(e_new)
        e_out[order] = e_new
        return x_new, e_out

    p = params
    c1e = [(np.asarray(W), np.asarray(b)) for W, b in p['c1_edge']]
    c1n = [(np.asarray(W), np.asarray(b)) for W, b in p['c1_node']]
    c2e = [(np.asarray(W), np.asarray(b)) for W, b in p['c2_edge']]
    c2n = [(np.asarray(W), np.asarray(b)) for W, b in p['c2_node']]
    c3e = [(np.asarray(W), np.asarray(b)) for W, b in p['c3_edge']]
    c3n = [(np.asarray(W), np.asarray(b)) for W, b in p['c3_node']]

    x, e = conv(x_node, x_edge, c1e, c1n, 2, 2)
    x, e = conv(x, e, c2e, c2n, 2, 2)
    x, e = conv(x, e, c3e, c3n, 3, 3)
    return (x, e)
